# revision 51
# baseline (speedup 1.0000x reference)
"""NCE classifier scores kernel for Trainium2 (8 NeuronCores, SPMD).

scores = -(||q||^2 + ||p||^2 - 2 q.p) / T  for q = x[:8192], p = x[8192:].

Sharding: 2D grid (2 query groups x 4 proto groups). Core c = (qi, pj)
computes the [4096, 2048] output block for query rows qi*4096.. and proto
rows pj*2048.. — this minimizes per-core HBM traffic (24 MB in + 32 MB out
vs 36 + 32 for pure query-parallel).

Per-core device kernel (fp8 DoubleRow matmuls):
  - Both operands are scaled by alpha = sqrt(2/T) (computed on device) and
    cast to fp8e4, so PSUM accumulates (2/T) * q.p directly while keeping
    values in e4m3's sweet range (~N(0, 0.177)).
  - Q chunk 0 loads as four f32 row-slabs on the sync+scalar HWDGE rings
    (issues at t~0, full rate) and PE-transposes directly in fp32; later q
    chunks use gpsimd SWDGE cast-DMAs (f32->bf16 conversion inline in the
    DMA datapath, ~half-rate source side but off the critical path; chunk 1
    split in halves so Q7 descriptor generation pipelines with the
    transfer, issued after the gpsimd const block so identities/alpha are
    ready early). P chunks load as f32 HWDGE halves with a DVE bf16 copy;
    ScalarE does Square+accum row norms.
  - P phase: 4 proto chunks -> PE-transposed resident fp8 P^T
    [128, 8k, 512] + a ones-matmul broadcast of ||p||^2/T into [128, 512]
    correction tiles.
  - Q phase: 8 chunks of 512 query rows through the same transpose path;
    DoubleRow matmuls (K=256 per instruction, 4 per 1024-deep dot)
    accumulate q.p into PSUM; one VectorE scalar_tensor_tensor applies both
    rank-1 corrections; 1 MB output DMAs alternate the sync/scalar rings
    (the last chunk splits per-256KB-subtile to shorten the tail drain).
  - Slow fp32 zero-matmul "warm keepers" fill the input-bound prologue
    waits so the PE HAM clock gate stays at 8/8 (2.4 GHz) instead of
    re-throttling to 4/8 across every early stall.
"""

import os
import sys

import numpy as np

NUM_BATCH = 8192
NUM_PROTO = 8192
DIM = 1024
N_CORES = 8
QG = 2  # query groups
PG = 4  # proto groups
QPC = NUM_BATCH // QG  # query rows per core: 4096
PPC = NUM_PROTO // PG  # proto rows per core: 2048
P = 128  # partitions
CH = 512  # chunk of rows (both q and p streaming granularity)
NPCH = PPC // CH  # 4 proto chunks
NQCH = QPC // CH  # 8 query chunks
CPT = CH // P  # 4 row tiles per chunk
KT = DIM // P  # 8 contraction subtiles of 128
KD = KT // 2  # 4 DoubleRow matmuls per 1024-deep contraction


def _install_axon_hooks_shim():
    """Provide antenv.axon_hooks (NTFF profiling hook) if the image lacks it."""
    try:
        import antenv.axon_hooks  # noqa: F401

        return
    except ImportError:
        pass
    import contextlib
    import ctypes
    import types

    mod = types.ModuleType("antenv.axon_hooks")
    _state = {"hook": None}
    mod.set_axon_ntff_profile_hook = lambda h: _state.__setitem__("hook", h)
    mod.get_axon_ntff_profile_hook = lambda: _state["hook"]
    sys.modules["antenv.axon_hooks"] = mod
    try:
        import antenv

        antenv.axon_hooks = mod
    except ImportError:
        pass
    so_path = "/opt/axon/libaxon_pjrt.so"
    if not os.path.exists(so_path):
        return
    try:
        lib = ctypes.CDLL(so_path)
        if not hasattr(lib, "axon_start_nrt_profile"):
            return
        lib.axon_start_nrt_profile.argtypes = [
            ctypes.POINTER(ctypes.c_int64),
            ctypes.c_size_t,
        ]
        lib.axon_start_nrt_profile.restype = ctypes.c_int64
        lib.axon_stop_nrt_profile.argtypes = [ctypes.c_char_p]
        lib.axon_stop_nrt_profile.restype = ctypes.c_int64

        @contextlib.contextmanager
        def _hook(output_dir, device_ids):
            import jax

            jax.devices()
            if device_ids:
                ids = (ctypes.c_int64 * len(device_ids))(*device_ids)
                rc = lib.axon_start_nrt_profile(ids, len(device_ids))
            else:
                rc = lib.axon_start_nrt_profile(None, 0)
            if rc != 0:
                raise RuntimeError(f"axon_start_nrt_profile rc={rc}")
            try:
                yield
            finally:
                n = lib.axon_stop_nrt_profile(str(output_dir).encode())
                print(f"profile: {n} file(s) written to {output_dir}")

        mod.set_axon_ntff_profile_hook(_hook)
    except OSError:
        pass


_NC_CACHE = {}


def _build_nc():
    if "nc" in _NC_CACHE:
        return _NC_CACHE["nc"]
    from contextlib import ExitStack

    import concourse.bacc as bacc
    import concourse.mybir as mybir
    import concourse.tile as tile
    from concourse.masks import make_identity

    F32 = mybir.dt.float32
    F32R = mybir.dt.float32r
    BF16 = mybir.dt.bfloat16
    FP8 = mybir.dt.float8e4
    SUB = mybir.AluOpType.subtract
    MULT = mybir.AluOpType.mult
    DR = mybir.MatmulPerfMode.DoubleRow

    nc = bacc.Bacc("TRN2", target_bir_lowering=False, debug=False)
    xq = nc.dram_tensor("xq", [QPC, DIM], F32, kind="ExternalInput").ap()
    xp = nc.dram_tensor("xp", [PPC, DIM], F32, kind="ExternalInput").ap()
    temp = nc.dram_tensor("temp", [1, 1], F32, kind="ExternalInput").ap()
    out = nc.dram_tensor("out", [QPC, PPC], F32, kind="ExternalOutput").ap()

    with tile.TileContext(nc) as tc:
        with ExitStack() as ctx:
            const = ctx.enter_context(tc.tile_pool(name="const", bufs=1))
            pnpool = ctx.enter_context(tc.tile_pool(name="pnpool", bufs=3))
            pbpool = ctx.enter_context(tc.tile_pool(name="pbpool", bufs=2))
            ptpool = ctx.enter_context(tc.tile_pool(name="ptpool", bufs=NPCH))
            qnpool = ctx.enter_context(tc.tile_pool(name="qnpool", bufs=4))
            qfpool = ctx.enter_context(tc.tile_pool(name="qfpool", bufs=1))
            qtpool = ctx.enter_context(tc.tile_pool(name="qtpool", bufs=2))
            bpool = ctx.enter_context(tc.tile_pool(name="bpool", bufs=8))
            rpool = ctx.enter_context(tc.tile_pool(name="rpool", bufs=2))
            tpool = ctx.enter_context(tc.tile_pool(name="tpool", bufs=2))
            opool = ctx.enter_context(tc.tile_pool(name="opool", bufs=5))
            psum_mm = ctx.enter_context(
                tc.tile_pool(name="psum_mm", bufs=4, space="PSUM")
            )
            psum_tr = ctx.enter_context(
                tc.tile_pool(name="psum_tr", bufs=3, space="PSUM")
            )
            psum_bc = ctx.enter_context(
                tc.tile_pool(name="psum_bc", bufs=1, space="PSUM")
            )

            # ---- inputs: SWDGE cast-DMAs (f32 -> bf16 conversion inline in
            # the DMA datapath) halve the logical input bytes. All on gpsimd,
            # interleaved q/p so the wavefront below always has work.
            pnat_tiles = {}
            qnat_tiles = {}
            qf_tiles = {}

            def dma_qf(c):
                qf = qfpool.tile([P, CPT, DIM], F32, tag="qf", name="qf")
                for j in range(CPT):
                    eng = nc.sync if j % 2 == 0 else nc.scalar
                    eng.dma_start(
                        qf[:, j, :], xq[c * CH + j * P : c * CH + (j + 1) * P, :]
                    )
                qf_tiles[c] = qf

            def dma_p(c):
                pnat = pnpool.tile([P, CPT, DIM], F32, tag="pnat", name="pnat")
                for h, eng in enumerate((nc.scalar, nc.sync)):
                    eng.dma_start(
                        pnat[:, h * 2 : (h + 1) * 2, :],
                        xp[
                            c * CH + h * 256 : c * CH + (h + 1) * 256, :
                        ].rearrange("(j p) d -> p j d", p=P),
                    )
                pnat_tiles[c] = pnat

            def dma_q(c, halves=False):
                qnat = qnpool.tile([P, CPT, DIM], BF16, tag="qnat", name="qnat")
                if halves:
                    # split the SWDGE cast-DMA so Q7 descriptor generation
                    # pipelines with the transfer (and with the other half)
                    for h in range(2):
                        nc.gpsimd.dma_start(
                            qnat[:, h * 2 : (h + 1) * 2, :],
                            xq[
                                c * CH + h * 256 : c * CH + (h + 1) * 256, :
                            ].rearrange("(j p) d -> p j d", p=P),
                        )
                else:
                    nc.gpsimd.dma_start(
                        qnat[:],
                        xq[c * CH : (c + 1) * CH, :].rearrange(
                            "(j p) d -> p j d", p=P
                        ),
                    )
                qnat_tiles[c] = qnat

            # temp is 4 bytes - issue it FIRST on a HWDGE queue so the alpha
            # chain does not stall behind megabytes of input DMAs.
            t11 = const.tile([1, 1], F32)
            nc.sync.dma_start(t11[:], temp[:])
            dma_qf(0)
            dma_p(0)
            dma_p(1)
            dma_p(2)
            dma_p(3)

            ident = const.tile([P, P], BF16)
            make_identity(nc, ident)
            identf = const.tile([P, P], F32)
            make_identity(nc, identf)
            zwarm = const.tile([P, CH], F32)
            nc.gpsimd.memset(zwarm[:], 0.0)

            def warm(n):
                # slow fp32 dummy matmuls (~1us each cold) that keep the PE
                # HAM activity window busy during input-bound prologue waits,
                # so real work afterwards runs at 2.4 GHz instead of 1.2.
                # They borrow psum_mm buffers (idle during the prologue).
                for _ in range(n):
                    wps = psum_bc.tile([P, CH], F32, tag="ps_b", name="warm")
                    nc.tensor.matmul(
                        wps[:], zwarm[:, :P], zwarm[:], start=True, stop=True
                    )
            ones_row_f = const.tile([1, P], F32)
            nc.gpsimd.memset(ones_row_f[:], 1.0)
            ones_row = ones_row_f.bitcast(F32R)

            # ---- temperature-derived scalars ----
            inv11 = const.tile([1, 1], F32)
            nc.vector.reciprocal(inv11[:], t11[:])
            al11 = const.tile([1, 1], F32)
            nc.vector.tensor_scalar(al11[:], inv11[:], 2.0, None, MULT)
            nc.scalar.activation(
                out=al11[:], in_=al11[:], func=mybir.ActivationFunctionType.Sqrt
            )
            invT = const.tile([P, 1], F32)
            nc.gpsimd.partition_broadcast(invT[:], inv11[:])
            alpha = const.tile([P, 1], F32)
            nc.gpsimd.partition_broadcast(alpha[:], al11[:])
            dma_q(1, halves=True)

            pts = [None] * NPCH
            psq_bs = [None] * NPCH

            def proc_p(c):
                """P chunk: transposes (PE) + fp8 scale-casts (DVE) first,
                then squares (ScalarE) and the psq broadcast chain."""
                pnat = pnat_tiles.pop(c)
                pnb = pbpool.tile([P, CPT, DIM], BF16, tag="pnb", name="pnb")
                nc.vector.tensor_copy(pnb[:], pnat[:])
                pt = ptpool.tile([P, KT, CH], FP8, tag="pt", name="pt")
                for k in range(KT):
                    pst = psum_tr.tile([P, CH], BF16, tag="pst", name="pst")
                    for j in range(CPT):
                        nc.tensor.transpose(
                            pst[:, j * P : (j + 1) * P],
                            pnb[:, j, k * P : (k + 1) * P],
                            ident[:],
                        )
                    nc.vector.tensor_scalar(pt[:, k, :], pst[:], alpha[:], None, MULT)
                pts[c] = pt

                psq4 = bpool.tile([P, CPT], F32, tag="psq4", name="psq4")
                for j in range(CPT):
                    trash = tpool.tile([P, DIM], BF16, tag="trash", name="trash")
                    nc.scalar.activation(
                        out=trash[:],
                        in_=pnat[:, j, :],
                        func=mybir.ActivationFunctionType.Square,
                        accum_out=psq4[:, j : j + 1],
                    )
                psq4s = bpool.tile([P, CPT], F32R, tag="psq4s", name="psq4s")
                nc.vector.tensor_scalar(psq4s[:], psq4[:], invT[:], None, MULT)
                psq_row = rpool.tile([1, CH], F32R, tag="psq_row", name="psq_row")
                for j in range(CPT):
                    nc.scalar.dma_start(
                        psq_row[:, j * P : (j + 1) * P], psq4s[:, j : j + 1]
                    )
                ps_b = psum_bc.tile([P, CH], F32, tag="ps_b", name="ps_b")
                nc.tensor.matmul(
                    ps_b[:], ones_row[:], psq_row[:], start=True, stop=True
                )
                psq_b = const.tile([P, CH], F32, name=f"psq_b{c}")
                nc.scalar.copy(psq_b[:], ps_b[:])
                psq_bs[c] = psq_b

            qts = [None] * NQCH
            qsqss = [None] * NQCH

            def prep_q(c):
                """Q chunk transposes (PE) + fp8 scale-casts (ScalarE -
                VectorE's in-order queue is backed up with STTs). Chunk 0 is
                loaded f32 on the fast HWDGE path and transposes in fp32."""
                if c == 0:
                    qnat, idt, pdt = qf_tiles[c], identf, F32
                else:
                    qnat, idt, pdt = qnat_tiles[c], ident, BF16
                qt = qtpool.tile([P, KT, CH], FP8, tag="qt", name="qt")
                for k in range(KT):
                    pst = psum_tr.tile([P, CH], pdt, tag="pst", name="pst")
                    for j in range(CPT):
                        nc.tensor.transpose(
                            pst[:, j * P : (j + 1) * P],
                            qnat[:, j, k * P : (k + 1) * P],
                            idt[:],
                        )
                    nc.scalar.activation(
                        out=qt[:, k, :],
                        in_=pst[:],
                        func=mybir.ActivationFunctionType.Copy,
                        scale=alpha[:],
                    )
                qts[c] = qt

            def sq_q(c):
                """Q chunk row squares on ScalarE."""
                qnat = qf_tiles.pop(c) if c == 0 else qnat_tiles.pop(c)
                qsq4 = bpool.tile([P, CPT], F32, tag="qsq4", name="qsq4")
                for j in range(CPT):
                    trash = tpool.tile([P, DIM], BF16, tag="trash", name="trash")
                    nc.scalar.activation(
                        out=trash[:],
                        in_=qnat[:, j, :],
                        func=mybir.ActivationFunctionType.Square,
                        accum_out=qsq4[:, j : j + 1],
                    )
                qsqs = bpool.tile([P, CPT], F32, tag="qsqs", name=f"qsqs{c}")
                nc.vector.tensor_scalar(qsqs[:], qsq4[:], invT[:], None, MULT)
                qsqss[c] = qsqs

            def mm_block(c, pc):
                """All matmuls+corrections of q-chunk c against proto chunk pc."""
                qt = qts[c]
                ost = opool.tile([P, CPT, CH], F32, tag="ost", name="ost")
                for j in range(CPT):
                    ps = psum_mm.tile([P, CH], F32, tag="mm", name="mm")
                    for kd in range(KD):
                        nc.tensor.matmul(
                            ps[:],
                            qt[:, 2 * kd : 2 * kd + 2, j * P : (j + 1) * P],
                            pts[pc][:, 2 * kd : 2 * kd + 2, :],
                            start=(kd == 0),
                            stop=(kd == KD - 1),
                            perf_mode=DR,
                        )
                    nc.vector.scalar_tensor_tensor(
                        out=ost[:, j, :],
                        in0=ps[:],
                        scalar=qsqss[c][:, j : j + 1],
                        in1=psq_bs[pc][:],
                        op0=SUB,
                        op1=SUB,
                    )
                if c == NQCH - 1:
                    for j in range(CPT):
                        oeng = nc.sync if j % 2 == 0 else nc.scalar
                        oeng.dma_start(
                            out[
                                c * CH + j * P : c * CH + (j + 1) * P,
                                pc * CH : (pc + 1) * CH,
                            ],
                            ost[:, j, :],
                        )
                else:
                    oeng = nc.sync if pc % 2 == 0 else nc.scalar
                    oeng.dma_start(
                        out[
                            c * CH : (c + 1) * CH, pc * CH : (pc + 1) * CH
                        ].rearrange("(j p) n -> p j n", p=P),
                        ost[:],
                    )

            # ---- wavefront emission: mm blocks ordered by input arrival so
            # the PE never waits on the full P panel before starting. ----
            warm(4)
            prep_q(0)
            sq_q(0)
            warm(4)
            proc_p(0)
            warm(2)
            proc_p(1)
            mm_block(0, 0)
            warm(3)
            proc_p(2)
            mm_block(0, 1)
            warm(3)
            proc_p(3)
            dma_q(2)
            mm_block(0, 2)
            prep_q(1)
            sq_q(1)
            dma_q(3)
            mm_block(0, 3)
            warm(2)
            for c in range(1, NQCH):
                if c + 3 < NQCH:
                    dma_q(c + 3)
                mm_block(c, 0)
                if c + 1 < NQCH:
                    prep_q(c + 1)
                    sq_q(c + 1)
                mm_block(c, 1)
                mm_block(c, 2)
                mm_block(c, 3)

    nc.compile()
    _NC_CACHE["nc"] = nc
    return nc


def _run(x, temperature, trace=False):
    _install_axon_hooks_shim()
    from concourse.bass_utils import run_bass_kernel_spmd

    nc = _build_nc()
    x = np.ascontiguousarray(np.asarray(x, dtype=np.float32))
    t = np.asarray(temperature, dtype=np.float32).reshape(1, 1)
    in_maps = []
    for c in range(N_CORES):
        qi, pj = divmod(c, PG)
        in_maps.append(
            {
                "xq": np.ascontiguousarray(x[qi * QPC : (qi + 1) * QPC]),
                "xp": np.ascontiguousarray(
                    x[NUM_BATCH + pj * PPC : NUM_BATCH + (pj + 1) * PPC]
                ),
                "temp": t,
            }
        )
    res = run_bass_kernel_spmd(
        nc,
        in_maps,
        core_ids=list(range(N_CORES)),
        trace=trace,
        trace_cores=[0] if trace else None,
    )
    full = np.empty((NUM_BATCH, NUM_PROTO), dtype=np.float32)
    for c in range(N_CORES):
        qi, pj = divmod(c, PG)
        full[qi * QPC : (qi + 1) * QPC, pj * PPC : (pj + 1) * PPC] = res.results[c][
            "out"
        ]
    return full, res


def kernel(x, temperature, num_batch):
    assert int(num_batch) == NUM_BATCH, f"kernel hardcoded for num_batch={NUM_BATCH}"
    x = np.asarray(x)
    assert x.shape == (NUM_BATCH + NUM_PROTO, DIM), x.shape
    out, _ = _run(x, temperature, trace=False)
    return out


# revision 52
# speedup vs baseline: 1.1172x; 1.1172x over previous
"""NCE classifier scores kernel for Trainium2 (8 NeuronCores, SPMD).

scores = -(||q||^2 + ||p||^2 - 2 q.p) / T  for q = x[:8192], p = x[8192:].

Sharding: 2D grid (2 query groups x 4 proto groups). Core c = (qi, pj)
computes the [4096, 2048] output block for query rows qi*4096.. and proto
rows pj*2048.. — this minimizes per-core HBM traffic (24 MB in + 32 MB out
vs 36 + 32 for pure query-parallel).

Per-core device kernel (fp8 DoubleRow matmuls):
  - Both operands are scaled by alpha = sqrt(2/T) (computed on device) and
    cast to fp8e4, so PSUM accumulates (2/T) * q.p directly while keeping
    values in e4m3's sweet range (~N(0, 0.177)).
  - Q chunk 0 loads as four f32 row-slabs on the sync+scalar HWDGE rings
    (issues at t~0, full rate) and PE-transposes directly in fp32; later q
    chunks use gpsimd SWDGE cast-DMAs (f32->bf16 conversion inline in the
    DMA datapath, ~half-rate source side but off the critical path; chunk 1
    split in halves so Q7 descriptor generation pipelines with the
    transfer, issued after the gpsimd const block so identities/alpha are
    ready early). P chunks load as f32 HWDGE halves with a DVE bf16 copy;
    ScalarE does Square+accum row norms.
  - P phase: 4 proto chunks -> PE-transposed resident fp8 P^T
    [128, 8k, 512] + a ones-matmul broadcast of ||p||^2/T into [128, 512]
    correction tiles.
  - Q phase: 8 chunks of 512 query rows through the same transpose path;
    DoubleRow matmuls (K=256 per instruction, 4 per 1024-deep dot)
    accumulate q.p into PSUM; one VectorE scalar_tensor_tensor applies both
    rank-1 corrections; 1 MB output DMAs alternate the sync/scalar rings
    (the last chunk splits per-256KB-subtile to shorten the tail drain).
  - Slow fp32 zero-matmul "warm keepers" fill the input-bound prologue
    waits so the PE HAM clock gate stays at 8/8 (2.4 GHz) instead of
    re-throttling to 4/8 across every early stall.
"""

import os
import sys

import numpy as np

NUM_BATCH = 8192
NUM_PROTO = 8192
DIM = 1024
N_CORES = 8
QG = 2  # query groups
PG = 4  # proto groups
QPC = NUM_BATCH // QG  # query rows per core: 4096
PPC = NUM_PROTO // PG  # proto rows per core: 2048
P = 128  # partitions
CH = 512  # chunk of rows (both q and p streaming granularity)
NPCH = PPC // CH  # 4 proto chunks
NQCH = QPC // CH  # 8 query chunks
CPT = CH // P  # 4 row tiles per chunk
KT = DIM // P  # 8 contraction subtiles of 128
KD = KT // 2  # 4 DoubleRow matmuls per 1024-deep contraction


def _install_axon_hooks_shim():
    """Provide antenv.axon_hooks (NTFF profiling hook) if the image lacks it."""
    try:
        import antenv.axon_hooks  # noqa: F401

        return
    except ImportError:
        pass
    import contextlib
    import ctypes
    import types

    mod = types.ModuleType("antenv.axon_hooks")
    _state = {"hook": None}
    mod.set_axon_ntff_profile_hook = lambda h: _state.__setitem__("hook", h)
    mod.get_axon_ntff_profile_hook = lambda: _state["hook"]
    sys.modules["antenv.axon_hooks"] = mod
    try:
        import antenv

        antenv.axon_hooks = mod
    except ImportError:
        pass
    so_path = "/opt/axon/libaxon_pjrt.so"
    if not os.path.exists(so_path):
        return
    try:
        lib = ctypes.CDLL(so_path)
        if not hasattr(lib, "axon_start_nrt_profile"):
            return
        lib.axon_start_nrt_profile.argtypes = [
            ctypes.POINTER(ctypes.c_int64),
            ctypes.c_size_t,
        ]
        lib.axon_start_nrt_profile.restype = ctypes.c_int64
        lib.axon_stop_nrt_profile.argtypes = [ctypes.c_char_p]
        lib.axon_stop_nrt_profile.restype = ctypes.c_int64

        @contextlib.contextmanager
        def _hook(output_dir, device_ids):
            import jax

            jax.devices()
            if device_ids:
                ids = (ctypes.c_int64 * len(device_ids))(*device_ids)
                rc = lib.axon_start_nrt_profile(ids, len(device_ids))
            else:
                rc = lib.axon_start_nrt_profile(None, 0)
            if rc != 0:
                raise RuntimeError(f"axon_start_nrt_profile rc={rc}")
            try:
                yield
            finally:
                n = lib.axon_stop_nrt_profile(str(output_dir).encode())
                print(f"profile: {n} file(s) written to {output_dir}")

        mod.set_axon_ntff_profile_hook(_hook)
    except OSError:
        pass


_NC_CACHE = {}


def _build_nc():
    if "nc" in _NC_CACHE:
        return _NC_CACHE["nc"]
    from contextlib import ExitStack

    import concourse.bacc as bacc
    import concourse.mybir as mybir
    import concourse.tile as tile
    from concourse.masks import make_identity

    F32 = mybir.dt.float32
    F32R = mybir.dt.float32r
    BF16 = mybir.dt.bfloat16
    FP8 = mybir.dt.float8e4
    SUB = mybir.AluOpType.subtract
    MULT = mybir.AluOpType.mult
    DR = mybir.MatmulPerfMode.DoubleRow

    nc = bacc.Bacc("TRN2", target_bir_lowering=False, debug=False)
    xq = nc.dram_tensor("xq", [QPC, DIM], F32, kind="ExternalInput").ap()
    xp = nc.dram_tensor("xp", [PPC, DIM], F32, kind="ExternalInput").ap()
    temp = nc.dram_tensor("temp", [1, 1], F32, kind="ExternalInput").ap()
    out = nc.dram_tensor("out", [QPC, PPC], F32, kind="ExternalOutput").ap()

    with tile.TileContext(nc) as tc:
        with ExitStack() as ctx:
            const = ctx.enter_context(tc.tile_pool(name="const", bufs=1))
            pnpool = ctx.enter_context(tc.tile_pool(name="pnpool", bufs=3))
            pbpool = ctx.enter_context(tc.tile_pool(name="pbpool", bufs=2))
            ptpool = ctx.enter_context(tc.tile_pool(name="ptpool", bufs=NPCH))
            qnpool = ctx.enter_context(tc.tile_pool(name="qnpool", bufs=4))
            qfpool = ctx.enter_context(tc.tile_pool(name="qfpool", bufs=1))
            qtpool = ctx.enter_context(tc.tile_pool(name="qtpool", bufs=2))
            bpool = ctx.enter_context(tc.tile_pool(name="bpool", bufs=8))
            rpool = ctx.enter_context(tc.tile_pool(name="rpool", bufs=2))
            tpool = ctx.enter_context(tc.tile_pool(name="tpool", bufs=2))
            opool = ctx.enter_context(tc.tile_pool(name="opool", bufs=5))
            psum_mm = ctx.enter_context(
                tc.tile_pool(name="psum_mm", bufs=4, space="PSUM")
            )
            psum_tr = ctx.enter_context(
                tc.tile_pool(name="psum_tr", bufs=3, space="PSUM")
            )
            psum_bc = ctx.enter_context(
                tc.tile_pool(name="psum_bc", bufs=1, space="PSUM")
            )

            # ---- inputs: SWDGE cast-DMAs (f32 -> bf16 conversion inline in
            # the DMA datapath) halve the logical input bytes. All on gpsimd,
            # interleaved q/p so the wavefront below always has work.
            pnat_tiles = {}
            qnat_tiles = {}
            qf_tiles = {}

            def dma_qf(c):
                qf = qfpool.tile([P, CPT, DIM], F32, tag="qf", name="qf")
                for j in range(CPT):
                    eng = nc.sync if j % 2 == 0 else nc.scalar
                    eng.dma_start(
                        qf[:, j, :], xq[c * CH + j * P : c * CH + (j + 1) * P, :]
                    )
                qf_tiles[c] = qf

            def dma_p(c):
                pnat = pnpool.tile([P, CPT, DIM], F32, tag="pnat", name="pnat")
                for h, eng in enumerate((nc.scalar, nc.sync)):
                    eng.dma_start(
                        pnat[:, h * 2 : (h + 1) * 2, :],
                        xp[
                            c * CH + h * 256 : c * CH + (h + 1) * 256, :
                        ].rearrange("(j p) d -> p j d", p=P),
                    )
                pnat_tiles[c] = pnat

            def dma_q(c, halves=False):
                qnat = qnpool.tile([P, CPT, DIM], BF16, tag="qnat", name="qnat")
                if halves:
                    # split the SWDGE cast-DMA so Q7 descriptor generation
                    # pipelines with the transfer (and with the other half)
                    for h in range(2):
                        nc.gpsimd.dma_start(
                            qnat[:, h * 2 : (h + 1) * 2, :],
                            xq[
                                c * CH + h * 256 : c * CH + (h + 1) * 256, :
                            ].rearrange("(j p) d -> p j d", p=P),
                        )
                else:
                    nc.gpsimd.dma_start(
                        qnat[:],
                        xq[c * CH : (c + 1) * CH, :].rearrange(
                            "(j p) d -> p j d", p=P
                        ),
                    )
                qnat_tiles[c] = qnat

            # temp is 4 bytes - issue it FIRST on a HWDGE queue so the alpha
            # chain does not stall behind megabytes of input DMAs.
            t11 = const.tile([1, 1], F32)
            nc.sync.dma_start(t11[:], temp[:])
            dma_qf(0)
            dma_p(0)
            dma_p(1)
            dma_p(2)
            dma_p(3)

            ident = const.tile([P, P], BF16)
            make_identity(nc, ident)
            identf = const.tile([P, P], F32)
            make_identity(nc, identf)
            zwarm = const.tile([P, CH], F32)
            nc.gpsimd.memset(zwarm[:], 0.0)

            def warm(n):
                # slow fp32 dummy matmuls (~1us each cold) that keep the PE
                # HAM activity window busy during input-bound prologue waits,
                # so real work afterwards runs at 2.4 GHz instead of 1.2.
                # They borrow psum_mm buffers (idle during the prologue).
                for _ in range(n):
                    wps = psum_bc.tile([P, CH], F32, tag="ps_b", name="warm")
                    nc.tensor.matmul(
                        wps[:], zwarm[:, :P], zwarm[:], start=True, stop=True
                    )
            ones_row_f = const.tile([1, P], F32)
            nc.gpsimd.memset(ones_row_f[:], 1.0)
            ones_row = ones_row_f.bitcast(F32R)

            # ---- temperature-derived scalars ----
            inv11 = const.tile([1, 1], F32)
            nc.vector.reciprocal(inv11[:], t11[:])
            al11 = const.tile([1, 1], F32)
            nc.vector.tensor_scalar(al11[:], inv11[:], 2.0, None, MULT)
            nc.scalar.activation(
                out=al11[:], in_=al11[:], func=mybir.ActivationFunctionType.Sqrt
            )
            invT = const.tile([P, 1], F32)
            nc.gpsimd.partition_broadcast(invT[:], inv11[:])
            alpha = const.tile([P, 1], F32)
            nc.gpsimd.partition_broadcast(alpha[:], al11[:])
            dma_q(1, halves=True)

            pts = [None] * NPCH
            psq_bs = [None] * NPCH

            def proc_p(c):
                """P chunk: transposes (PE) + fp8 scale-casts (DVE) first,
                then squares (ScalarE) and the psq broadcast chain."""
                pnat = pnat_tiles.pop(c)
                pnb = pbpool.tile([P, CPT, DIM], BF16, tag="pnb", name="pnb")
                nc.vector.tensor_copy(pnb[:], pnat[:])
                pt = ptpool.tile([P, KT, CH], FP8, tag="pt", name="pt")
                for k in range(KT):
                    pst = psum_tr.tile([P, CH], BF16, tag="pst", name="pst")
                    for j in range(CPT):
                        nc.tensor.transpose(
                            pst[:, j * P : (j + 1) * P],
                            pnb[:, j, k * P : (k + 1) * P],
                            ident[:],
                        )
                    nc.vector.tensor_scalar(pt[:, k, :], pst[:], alpha[:], None, MULT)
                pts[c] = pt

                psq4 = bpool.tile([P, CPT], F32, tag="psq4", name="psq4")
                for j in range(CPT):
                    trash = tpool.tile([P, DIM], BF16, tag="trash", name="trash")
                    nc.scalar.activation(
                        out=trash[:],
                        in_=pnat[:, j, :],
                        func=mybir.ActivationFunctionType.Square,
                        accum_out=psq4[:, j : j + 1],
                    )
                psq4s = bpool.tile([P, CPT], F32R, tag="psq4s", name="psq4s")
                nc.vector.tensor_scalar(psq4s[:], psq4[:], invT[:], None, MULT)
                psq_row = rpool.tile([1, CH], F32R, tag="psq_row", name="psq_row")
                for j in range(CPT):
                    nc.scalar.dma_start(
                        psq_row[:, j * P : (j + 1) * P], psq4s[:, j : j + 1]
                    )
                ps_b = psum_bc.tile([P, CH], F32, tag="ps_b", name="ps_b")
                nc.tensor.matmul(
                    ps_b[:], ones_row[:], psq_row[:], start=True, stop=True
                )
                psq_b = const.tile([P, CH], F32, name=f"psq_b{c}")
                nc.scalar.copy(psq_b[:], ps_b[:])
                psq_bs[c] = psq_b

            qts = [None] * NQCH
            qsqss = [None] * NQCH

            def prep_q(c):
                """Q chunk transposes (PE) + fp8 scale-casts (ScalarE -
                VectorE's in-order queue is backed up with STTs). Chunk 0 is
                loaded f32 on the fast HWDGE path and transposes in fp32."""
                if c == 0:
                    qnat, idt, pdt = qf_tiles[c], identf, F32
                else:
                    qnat, idt, pdt = qnat_tiles[c], ident, BF16
                qt = qtpool.tile([P, KT, CH], FP8, tag="qt", name="qt")
                for k in range(KT):
                    pst = psum_tr.tile([P, CH], pdt, tag="pst", name="pst")
                    for j in range(CPT):
                        nc.tensor.transpose(
                            pst[:, j * P : (j + 1) * P],
                            qnat[:, j, k * P : (k + 1) * P],
                            idt[:],
                        )
                    nc.scalar.activation(
                        out=qt[:, k, :],
                        in_=pst[:],
                        func=mybir.ActivationFunctionType.Copy,
                        scale=alpha[:],
                    )
                qts[c] = qt

            def sq_q(c):
                """Q chunk row squares on ScalarE."""
                qnat = qf_tiles.pop(c) if c == 0 else qnat_tiles.pop(c)
                qsq4 = bpool.tile([P, CPT], F32, tag="qsq4", name="qsq4")
                for j in range(CPT):
                    trash = tpool.tile([P, DIM], BF16, tag="trash", name="trash")
                    nc.scalar.activation(
                        out=trash[:],
                        in_=qnat[:, j, :],
                        func=mybir.ActivationFunctionType.Square,
                        accum_out=qsq4[:, j : j + 1],
                    )
                qsqs = bpool.tile([P, CPT], F32, tag="qsqs", name=f"qsqs{c}")
                nc.vector.tensor_scalar(qsqs[:], qsq4[:], invT[:], None, MULT)
                qsqss[c] = qsqs

            def mm_block(c, pc):
                """All matmuls+corrections of q-chunk c against proto chunk pc."""
                qt = qts[c]
                ost = opool.tile([P, CPT, CH], F32, tag="ost", name="ost")
                for j in range(CPT):
                    ps = psum_mm.tile([P, CH], F32, tag="mm", name="mm")
                    for kd in range(KD):
                        nc.tensor.matmul(
                            ps[:],
                            qt[:, 2 * kd : 2 * kd + 2, j * P : (j + 1) * P],
                            pts[pc][:, 2 * kd : 2 * kd + 2, :],
                            start=(kd == 0),
                            stop=(kd == KD - 1),
                            perf_mode=DR,
                        )
                    nc.vector.scalar_tensor_tensor(
                        out=ost[:, j, :],
                        in0=ps[:],
                        scalar=qsqss[c][:, j : j + 1],
                        in1=psq_bs[pc][:],
                        op0=SUB,
                        op1=SUB,
                    )
                if c == NQCH - 1:
                    for j in range(CPT):
                        oeng = nc.sync if j % 2 == 0 else nc.scalar
                        oeng.dma_start(
                            out[
                                c * CH + j * P : c * CH + (j + 1) * P,
                                pc * CH : (pc + 1) * CH,
                            ],
                            ost[:, j, :],
                        )
                else:
                    oeng = nc.sync if pc % 2 == 0 else nc.scalar
                    oeng.dma_start(
                        out[
                            c * CH : (c + 1) * CH, pc * CH : (pc + 1) * CH
                        ].rearrange("(j p) n -> p j n", p=P),
                        ost[:],
                    )

            # ---- wavefront emission: mm blocks ordered by input arrival so
            # the PE never waits on the full P panel before starting. ----
            warm(4)
            prep_q(0)
            sq_q(0)
            warm(4)
            proc_p(0)
            warm(2)
            proc_p(1)
            mm_block(0, 0)
            warm(3)
            proc_p(2)
            mm_block(0, 1)
            warm(3)
            proc_p(3)
            dma_q(2)
            mm_block(0, 2)
            prep_q(1)
            sq_q(1)
            dma_q(3)
            mm_block(0, 3)
            warm(2)
            for c in range(1, NQCH):
                if c + 3 < NQCH:
                    dma_q(c + 3)
                mm_block(c, 0)
                mm_block(c, 1)
                if c + 1 < NQCH:
                    prep_q(c + 1)
                    sq_q(c + 1)
                mm_block(c, 2)
                mm_block(c, 3)

    nc.compile()
    _NC_CACHE["nc"] = nc
    return nc


def _run(x, temperature, trace=False):
    _install_axon_hooks_shim()
    from concourse.bass_utils import run_bass_kernel_spmd

    nc = _build_nc()
    x = np.ascontiguousarray(np.asarray(x, dtype=np.float32))
    t = np.asarray(temperature, dtype=np.float32).reshape(1, 1)
    in_maps = []
    for c in range(N_CORES):
        qi, pj = divmod(c, PG)
        in_maps.append(
            {
                "xq": np.ascontiguousarray(x[qi * QPC : (qi + 1) * QPC]),
                "xp": np.ascontiguousarray(
                    x[NUM_BATCH + pj * PPC : NUM_BATCH + (pj + 1) * PPC]
                ),
                "temp": t,
            }
        )
    res = run_bass_kernel_spmd(
        nc,
        in_maps,
        core_ids=list(range(N_CORES)),
        trace=trace,
        trace_cores=[0] if trace else None,
    )
    full = np.empty((NUM_BATCH, NUM_PROTO), dtype=np.float32)
    for c in range(N_CORES):
        qi, pj = divmod(c, PG)
        full[qi * QPC : (qi + 1) * QPC, pj * PPC : (pj + 1) * PPC] = res.results[c][
            "out"
        ]
    return full, res


def kernel(x, temperature, num_batch):
    assert int(num_batch) == NUM_BATCH, f"kernel hardcoded for num_batch={NUM_BATCH}"
    x = np.asarray(x)
    assert x.shape == (NUM_BATCH + NUM_PROTO, DIM), x.shape
    out, _ = _run(x, temperature, trace=False)
    return out
